# revision 4
# baseline (speedup 1.0000x reference)
"""Grouped-experts SwiGLU MLP kernel for Trainium2, expert-parallel over 8 cores.

Per core (one expert): out = (silu(x @ Wg) * (x @ Wu)) @ Wd
  x  [2048 tok, 2048 dim]   Wg,Wu [2048 dim, 5632 hid]   Wd [5632 hid, 2048 dim]

Layout trick: host supplies x transposed (xT [dim, tok]).  Phase A computes the
intermediate in transposed layout hT [hid, tok] (stationary = Wg/Wu k-tile,
moving = xT), so phase B can use hT tiles as the stationary operand directly
against Wd (natural layout) and produce out [tok, dim] in natural orientation.
No on-device transposes anywhere.

Numerics: bf16 operands, fp32 PSUM accumulation, fp32 output
(simulated end-to-end L2 rel err vs fp32 ≈ 4e-3).
"""

import sys

import numpy as np

if "/opt/trn_rl_repo" not in sys.path:
    sys.path.insert(0, "/opt/trn_rl_repo")

import ml_dtypes

P = 128
DIM = 2048
HID = 5632
CAP = 2048  # token capacity per expert
NEXP = 8
KD = DIM // P  # 16 k-tiles over dim
KH = HID // P  # 44 k-tiles over hid
NTOK = 512  # phase A tok group (PSUM bank = 512 fp32)
HALF = 1024  # tok processed per outer iteration
ND = 512  # phase B dim chunk

BF16 = ml_dtypes.bfloat16

_NC_CACHE = {}


def _build_nc():
    import concourse.tile as tile
    from concourse import bacc, mybir

    f32 = mybir.dt.float32
    bf16 = mybir.dt.bfloat16
    Silu = mybir.ActivationFunctionType.Silu

    nc = bacc.Bacc("TRN2", target_bir_lowering=False, debug=False)
    xT = nc.dram_tensor("xT", [DIM, CAP], bf16, kind="ExternalInput").ap()
    wg = nc.dram_tensor("wg", [DIM, HID], bf16, kind="ExternalInput").ap()
    wu = nc.dram_tensor("wu", [DIM, HID], bf16, kind="ExternalInput").ap()
    wd = nc.dram_tensor("wd", [HID, DIM], bf16, kind="ExternalInput").ap()
    out = nc.dram_tensor("out", [CAP, DIM], f32, kind="ExternalOutput").ap()

    xTr = xT.rearrange("(ko p) t -> p ko t", p=P)  # [128, 16, 2048]
    wgr = wg.rearrange("(ko p) h -> p ko h", p=P)  # [128, 16, 5632]
    wur = wu.rearrange("(ko p) h -> p ko h", p=P)
    wdr = wd.rearrange("(ko p) d -> p ko d", p=P)  # [128, 44, 2048]

    with tile.TileContext(nc) as tc:
        with (
            tc.tile_pool(name="xp", bufs=1) as xp,
            tc.tile_pool(name="hp", bufs=1) as hp,
            tc.tile_pool(name="wab", bufs=2) as wab,
            tc.tile_pool(name="wdp", bufs=4) as wdp,
            tc.tile_pool(name="tmp", bufs=3) as tmps,
            tc.tile_pool(name="ev", bufs=4) as ev,
            tc.tile_pool(name="ps", bufs=2, space="PSUM") as psp,
            tc.tile_pool(name="pso", bufs=4, space="PSUM") as psop,
        ):
            for half in range(CAP // HALF):
                t0 = half * HALF
                xt = xp.tile([P, KD, HALF], bf16, tag="xt")
                nc.sync.dma_start(xt[:], xTr[:, :, t0 : t0 + HALF])
                ht = hp.tile([P, KH, HALF], bf16, tag="ht")

                # ---- Phase A: hT[hid, tok_half] = silu(x@Wg)*(x@Wu), transposed
                for m2 in range(KH // 2):  # pairs of 128-wide hid blocks
                    g2 = wab.tile([P, KD, 2 * P], bf16, tag="wg2")
                    nc.sync.dma_start(g2[:], wgr[:, :, m2 * 256 : (m2 + 1) * 256])
                    u2 = wab.tile([P, KD, 2 * P], bf16, tag="wu2")
                    nc.sync.dma_start(u2[:], wur[:, :, m2 * 256 : (m2 + 1) * 256])
                    for sm in range(2):
                        m = m2 * 2 + sm
                        for tg in range(HALF // NTOK):
                            ts_ = slice(tg * NTOK, (tg + 1) * NTOK)
                            pg = psp.tile([P, NTOK], f32, tag="pg")
                            pu = psp.tile([P, NTOK], f32, tag="pu")
                            for k in range(KD):
                                nc.tensor.matmul(
                                    pg,
                                    g2[:, k, sm * P : (sm + 1) * P],
                                    xt[:, k, ts_],
                                    start=(k == 0),
                                    stop=(k == KD - 1),
                                )
                            for k in range(KD):
                                nc.tensor.matmul(
                                    pu,
                                    u2[:, k, sm * P : (sm + 1) * P],
                                    xt[:, k, ts_],
                                    start=(k == 0),
                                    stop=(k == KD - 1),
                                )
                            sl = tmps.tile([P, NTOK], f32, tag="silu")
                            nc.scalar.activation(sl[:], pg[:], Silu)
                            nc.vector.tensor_mul(ht[:, m, ts_], sl[:], pu[:])

                # ---- Phase B: out[tok_half, dim] = hT.T @ Wd
                for dn in range(DIM // ND):
                    dslice = slice(dn * ND, (dn + 1) * ND)
                    for pset in range(2):  # 4 tok ptiles at a time (4 PSUM banks)
                        po = []
                        for _j in range(4):
                            po_t = psop.tile([P, ND], f32, tag="po", name=f"po{_j}")
                            po.append(po_t)
                        for k in range(KH):
                            wt = wdp.tile([P, ND], bf16, tag="wd")
                            nc.sync.dma_start(wt[:], wdr[:, k, dslice])
                            for j in range(4):
                                tp = pset * 4 + j
                                nc.tensor.matmul(
                                    po[j],
                                    ht[:, k, tp * P : (tp + 1) * P],
                                    wt[:],
                                    start=(k == 0),
                                    stop=(k == KH - 1),
                                )
                        for j in range(4):
                            tp = pset * 4 + j
                            o = ev.tile([P, ND], f32, tag="ev")
                            nc.vector.tensor_copy(o[:], po[j][:])
                            nc.sync.dma_start(
                                out[t0 + tp * P : t0 + (tp + 1) * P, dslice], o[:]
                            )
    nc.compile()
    return nc


def get_nc():
    if "nc" not in _NC_CACHE:
        _NC_CACHE["nc"] = _build_nc()
    return _NC_CACHE["nc"]


def make_in_maps(x, gate_proj, up_proj, down_proj, counts):
    offs = np.concatenate([[0], np.cumsum(counts)])
    in_maps = []
    for e in range(NEXP):
        xe = np.asarray(x[offs[e] : offs[e + 1]], dtype=np.float32)
        if xe.shape[0] < CAP:
            xe = np.concatenate(
                [xe, np.zeros((CAP - xe.shape[0], DIM), np.float32)], axis=0
            )
        in_maps.append(
            {
                "xT": xe.T.astype(BF16, order="C"),
                "wg": np.asarray(gate_proj[e], dtype=np.float32).astype(BF16),
                "wu": np.asarray(up_proj[e], dtype=np.float32).astype(BF16),
                "wd": np.asarray(down_proj[e], dtype=np.float32).astype(BF16),
            }
        )
    return in_maps


def kernel(x, gate_proj, up_proj, down_proj, num_tokens_per_expert, _run_kwargs=None):
    from concourse.bass_utils import run_bass_kernel_spmd

    x = np.asarray(x)
    counts = np.asarray(num_tokens_per_expert).astype(np.int64)
    assert counts.shape[0] == NEXP and counts.max() <= CAP

    in_maps = make_in_maps(x, gate_proj, up_proj, down_proj, counts)
    nc = get_nc()
    res = run_bass_kernel_spmd(
        nc, in_maps, core_ids=list(range(NEXP)), **(_run_kwargs or {})
    )
    outs = [res.results[e]["out"][: counts[e]] for e in range(NEXP)]
    full = np.concatenate(outs, axis=0).astype(np.float32)
    if _run_kwargs is not None:
        _run_kwargs["_results"] = res
    return full


# revision 5
# speedup vs baseline: 1.0501x; 1.0501x over previous
"""Grouped-experts SwiGLU MLP kernel for Trainium2, expert-parallel over 8 cores.

Per core (one expert): out = (silu(x @ Wg) * (x @ Wu)) @ Wd
  x  [2048 tok, 2048 dim]   Wg,Wu [2048 dim, 5632 hid]   Wd [5632 hid, 2048 dim]

Layout trick: host supplies x transposed (xT [dim, tok]).  Phase A computes the
intermediate in transposed layout hT [hid, tok] (stationary = Wg/Wu k-tile,
moving = xT), so phase B can use hT tiles as the stationary operand directly
against Wd (natural layout) and produce out [tok, dim] in natural orientation.
No on-device transposes anywhere.

DMA engine split: gpsimd (SWDGE) carries xT + the Wd stream (prefetchable
during phase A), sync (HWDGE) carries the Wg/Wu stream, scalar (HWDGE) carries
output stores.  One shared 8-bank PSUM pool: phase A keeps 4 gate/up pairs in
flight; phase B accumulates all 8 tok tiles concurrently so Wd streams once
per half.

Numerics: bf16 operands, fp32 PSUM accumulation, fp32 output
(measured end-to-end L2 rel err vs fp32 reference ≈ 4e-3).
"""

import sys

import numpy as np

if "/opt/trn_rl_repo" not in sys.path:
    sys.path.insert(0, "/opt/trn_rl_repo")

import ml_dtypes

P = 128
DIM = 2048
HID = 5632
CAP = 2048  # token capacity per expert
NEXP = 8
KD = DIM // P  # 16 k-tiles over dim
KH = HID // P  # 44 k-tiles over hid
NTOK = 512  # phase A tok group (PSUM bank = 512 fp32)
HALF = 1024  # tok processed per outer iteration
ND = 512  # phase B dim chunk
KC = 11  # phase B k-tiles per Wd DMA chunk (44 = 4*11)

BF16 = ml_dtypes.bfloat16

_NC_CACHE = {}


def _build_nc():
    import concourse.tile as tile
    from concourse import bacc, mybir

    f32 = mybir.dt.float32
    bf16 = mybir.dt.bfloat16
    Silu = mybir.ActivationFunctionType.Silu

    nc = bacc.Bacc("TRN2", target_bir_lowering=False, debug=False)
    xT = nc.dram_tensor("xT", [DIM, CAP], bf16, kind="ExternalInput").ap()
    wg = nc.dram_tensor("wg", [DIM, HID], bf16, kind="ExternalInput").ap()
    wu = nc.dram_tensor("wu", [DIM, HID], bf16, kind="ExternalInput").ap()
    wd = nc.dram_tensor("wd", [HID, DIM], bf16, kind="ExternalInput").ap()
    out = nc.dram_tensor("out", [CAP, DIM], f32, kind="ExternalOutput").ap()

    xTr = xT.rearrange("(ko p) t -> p ko t", p=P)  # [128, 16, 2048]
    wgr = wg.rearrange("(ko p) h -> p ko h", p=P)  # [128, 16, 5632]
    wur = wu.rearrange("(ko p) h -> p ko h", p=P)
    wdr = wd.rearrange("(ko p) d -> p ko d", p=P)  # [128, 44, 2048]

    with tile.TileContext(nc) as tc:
        with (
            tc.tile_pool(name="xp", bufs=2) as xp,
            tc.tile_pool(name="hp", bufs=1) as hp,
            tc.tile_pool(name="wab", bufs=2) as wab,
            tc.tile_pool(name="wdp", bufs=2) as wdp,
            tc.tile_pool(name="tmp", bufs=2) as tmps,
            tc.tile_pool(name="ev", bufs=3) as ev,
            tc.tile_pool(name="ps", bufs=8, space="PSUM") as psp,
        ):
            for half in range(CAP // HALF):
                t0 = half * HALF
                # xT half loaded as two 512-tok tiles so the first matmul
                # group only waits on 2.1 MB.
                xts = []
                for tg in range(HALF // NTOK):
                    xt_t = xp.tile([P, KD, NTOK], bf16, tag="xt", name=f"xt{tg}")
                    nc.gpsimd.dma_start(
                        xt_t[:], xTr[:, :, t0 + tg * NTOK : t0 + (tg + 1) * NTOK]
                    )
                    xts.append(xt_t)
                ht = hp.tile([P, KH, HALF], bf16, tag="ht")

                # ---- Phase A: hT[hid, tok_half] = silu(x@Wg)*(x@Wu), transposed
                for m2 in range(KH // 2):  # pairs of 128-wide hid blocks
                    g2 = wab.tile([P, KD, 2 * P], bf16, tag="wg2")
                    nc.sync.dma_start(g2[:], wgr[:, :, m2 * 256 : (m2 + 1) * 256])
                    u2 = wab.tile([P, KD, 2 * P], bf16, tag="wu2")
                    nc.sync.dma_start(u2[:], wur[:, :, m2 * 256 : (m2 + 1) * 256])
                    for sm in range(2):
                        m = m2 * 2 + sm
                        for tg in range(HALF // NTOK):
                            ts_ = slice(tg * NTOK, (tg + 1) * NTOK)
                            pg = psp.tile([P, NTOK], f32, tag="ps", name="pg")
                            pu = psp.tile([P, NTOK], f32, tag="ps", name="pu")
                            for k in range(KD):
                                nc.tensor.matmul(
                                    pg,
                                    g2[:, k, sm * P : (sm + 1) * P],
                                    xts[tg][:, k, :],
                                    start=(k == 0),
                                    stop=(k == KD - 1),
                                )
                            for k in range(KD):
                                nc.tensor.matmul(
                                    pu,
                                    u2[:, k, sm * P : (sm + 1) * P],
                                    xts[tg][:, k, :],
                                    start=(k == 0),
                                    stop=(k == KD - 1),
                                )
                            sl = tmps.tile([P, NTOK], f32, tag="silu")
                            nc.scalar.activation(sl[:], pg[:], Silu)
                            nc.vector.tensor_mul(ht[:, m, ts_], sl[:], pu[:])

                # ---- Phase B: out[tok_half, dim] = hT.T @ Wd
                # All 8 tok ptiles accumulate concurrently (8 PSUM banks), so
                # Wd streams exactly once per half, in 1.4 MB chunks on the
                # gpsimd queue (prefetched during phase A).
                for dn in range(DIM // ND):
                    dslice = slice(dn * ND, (dn + 1) * ND)
                    po = []
                    for j in range(8):
                        po_t = psp.tile([P, ND], f32, tag="ps", name=f"po{j}")
                        po.append(po_t)
                    for kc in range(KH // KC):
                        wt = wdp.tile([P, KC, ND], bf16, tag="wd")
                        nc.gpsimd.dma_start(
                            wt[:], wdr[:, kc * KC : (kc + 1) * KC, dslice]
                        )
                        for kk in range(KC):
                            k = kc * KC + kk
                            for j in range(8):
                                nc.tensor.matmul(
                                    po[j],
                                    ht[:, k, j * P : (j + 1) * P],
                                    wt[:, kk, :],
                                    start=(k == 0),
                                    stop=(k == KH - 1),
                                )
                    for j in range(8):
                        o = ev.tile([P, ND], f32, tag="ev")
                        nc.vector.tensor_copy(o[:], po[j][:])
                        nc.scalar.dma_start(
                            out[t0 + j * P : t0 + (j + 1) * P, dslice], o[:]
                        )
    nc.compile()
    return nc


def get_nc():
    if "nc" not in _NC_CACHE:
        _NC_CACHE["nc"] = _build_nc()
    return _NC_CACHE["nc"]


def make_in_maps(x, gate_proj, up_proj, down_proj, counts):
    offs = np.concatenate([[0], np.cumsum(counts)])
    in_maps = []
    for e in range(NEXP):
        xe = np.asarray(x[offs[e] : offs[e + 1]], dtype=np.float32)
        if xe.shape[0] < CAP:
            xe = np.concatenate(
                [xe, np.zeros((CAP - xe.shape[0], DIM), np.float32)], axis=0
            )
        in_maps.append(
            {
                "xT": xe.T.astype(BF16, order="C"),
                "wg": np.asarray(gate_proj[e], dtype=np.float32).astype(BF16),
                "wu": np.asarray(up_proj[e], dtype=np.float32).astype(BF16),
                "wd": np.asarray(down_proj[e], dtype=np.float32).astype(BF16),
            }
        )
    return in_maps


def kernel(x, gate_proj, up_proj, down_proj, num_tokens_per_expert, _run_kwargs=None):
    from concourse.bass_utils import run_bass_kernel_spmd

    x = np.asarray(x)
    counts = np.asarray(num_tokens_per_expert).astype(np.int64)
    assert counts.shape[0] == NEXP and counts.max() <= CAP

    in_maps = make_in_maps(x, gate_proj, up_proj, down_proj, counts)
    nc = get_nc()
    res = run_bass_kernel_spmd(
        nc, in_maps, core_ids=list(range(NEXP)), **(_run_kwargs or {})
    )
    outs = [res.results[e]["out"][: counts[e]] for e in range(NEXP)]
    full = np.concatenate(outs, axis=0).astype(np.float32)
    if _run_kwargs is not None:
        _run_kwargs["_results"] = res
    return full


# revision 7
# speedup vs baseline: 1.0580x; 1.0075x over previous
"""Grouped-experts SwiGLU MLP kernel for Trainium2, expert-parallel over 8 cores.

Per core (one expert): out = (silu(x @ Wg) * (x @ Wu)) @ Wd
  x  [2048 tok, 2048 dim]   Wg,Wu [2048 dim, 5632 hid]   Wd [5632 hid, 2048 dim]

Layout trick: host supplies x transposed (xT [dim, tok]).  Phase A computes the
intermediate in transposed layout hT [hid, tok] (stationary = Wg/Wu k-tile,
moving = xT), so phase B can use hT tiles as the stationary operand directly
against Wd (natural layout) and produce out [tok, dim] in natural orientation.
No on-device transposes anywhere.

DMA engine split: gpsimd (SWDGE) carries xT + the Wd stream (prefetchable
during phase A), sync (HWDGE) carries the Wg/Wu stream, scalar (HWDGE) carries
output stores.  One shared 8-bank PSUM pool: phase A keeps 4 gate/up pairs in
flight; phase B accumulates all 8 tok tiles concurrently so Wd streams once
per half.

Numerics: bf16 operands, fp32 PSUM accumulation, fp32 output
(measured end-to-end L2 rel err vs fp32 reference ≈ 4e-3).
"""

import sys

import numpy as np

if "/opt/trn_rl_repo" not in sys.path:
    sys.path.insert(0, "/opt/trn_rl_repo")

import ml_dtypes

P = 128
DIM = 2048
HID = 5632
CAP = 2048  # token capacity per expert
NEXP = 8
KD = DIM // P  # 16 k-tiles over dim
KH = HID // P  # 44 k-tiles over hid
NTOK = 512  # phase A tok group (PSUM bank = 512 fp32)
HALF = 1024  # tok processed per outer iteration
ND = 512  # phase B dim chunk
KC = 11  # phase B k-tiles per Wd DMA chunk (44 = 4*11)

BF16 = ml_dtypes.bfloat16

_NC_CACHE = {}


def _build_nc():
    import concourse.tile as tile
    from concourse import bacc, mybir

    f32 = mybir.dt.float32
    bf16 = mybir.dt.bfloat16
    Silu = mybir.ActivationFunctionType.Silu

    nc = bacc.Bacc("TRN2", target_bir_lowering=False, debug=False)
    xT = nc.dram_tensor("xT", [DIM, CAP], bf16, kind="ExternalInput").ap()
    wg = nc.dram_tensor("wg", [DIM, HID], bf16, kind="ExternalInput").ap()
    wu = nc.dram_tensor("wu", [DIM, HID], bf16, kind="ExternalInput").ap()
    wd = nc.dram_tensor("wd", [HID, DIM], bf16, kind="ExternalInput").ap()
    out = nc.dram_tensor("out", [CAP, DIM], f32, kind="ExternalOutput").ap()

    xTr = xT.rearrange("(ko p) t -> p ko t", p=P)  # [128, 16, 2048]
    wgr = wg.rearrange("(ko p) h -> p ko h", p=P)  # [128, 16, 5632]
    wur = wu.rearrange("(ko p) h -> p ko h", p=P)
    wdr = wd.rearrange("(ko p) d -> p ko d", p=P)  # [128, 44, 2048]

    with tile.TileContext(nc) as tc:
        with (
            tc.tile_pool(name="xp", bufs=2) as xp,
            tc.tile_pool(name="hp", bufs=1) as hp,
            tc.tile_pool(name="wab", bufs=3) as wab,
            tc.tile_pool(name="wdp", bufs=3) as wdp,
            tc.tile_pool(name="tmp", bufs=2) as tmps,
            tc.tile_pool(name="ev", bufs=4) as ev,
            tc.tile_pool(name="ps", bufs=8, space="PSUM") as psp,
        ):
            for half in range(CAP // HALF):
                t0 = half * HALF
                # xT half loaded as two 512-tok tiles on separate DMA queues so
                # the first matmul group only waits on 2.1 MB.
                xts = []
                for tg in range(HALF // NTOK):
                    xt_t = xp.tile([P, KD, NTOK], bf16, tag="xt", name=f"xt{tg}")
                    eng = nc.scalar if tg == 0 else nc.gpsimd
                    eng.dma_start(
                        xt_t[:], xTr[:, :, t0 + tg * NTOK : t0 + (tg + 1) * NTOK]
                    )
                    xts.append(xt_t)
                ht = hp.tile([P, KH, HALF], bf16, tag="ht")

                # ---- Phase A: hT[hid, tok_half] = silu(x@Wg)*(x@Wu), transposed
                for m in range(KH):  # 128-wide hid blocks
                    g1 = wab.tile([P, KD, P], bf16, tag="wg1")
                    nc.sync.dma_start(g1[:], wgr[:, :, m * P : (m + 1) * P])
                    u1 = wab.tile([P, KD, P], bf16, tag="wu1")
                    nc.sync.dma_start(u1[:], wur[:, :, m * P : (m + 1) * P])
                    for tg in range(HALF // NTOK):
                        ts_ = slice(tg * NTOK, (tg + 1) * NTOK)
                        pg = psp.tile([P, NTOK], f32, tag="ps", name="pg")
                        pu = psp.tile([P, NTOK], f32, tag="ps", name="pu")
                        for k in range(KD):
                            nc.tensor.matmul(
                                pg,
                                g1[:, k, :],
                                xts[tg][:, k, :],
                                start=(k == 0),
                                stop=(k == KD - 1),
                            )
                        for k in range(KD):
                            nc.tensor.matmul(
                                pu,
                                u1[:, k, :],
                                xts[tg][:, k, :],
                                start=(k == 0),
                                stop=(k == KD - 1),
                            )
                        sl = tmps.tile([P, NTOK], f32, tag="silu")
                        nc.scalar.activation(sl[:], pg[:], Silu)
                        nc.vector.tensor_mul(ht[:, m, ts_], sl[:], pu[:])

                # ---- Phase B: out[tok_half, dim] = hT.T @ Wd
                # All 8 tok ptiles accumulate concurrently (8 PSUM banks), so
                # Wd streams exactly once per half, in 1.4 MB chunks on the
                # gpsimd queue (prefetched during phase A).
                for dn in range(DIM // ND):
                    dslice = slice(dn * ND, (dn + 1) * ND)
                    po = []
                    for j in range(8):
                        po_t = psp.tile([P, ND], f32, tag="ps", name=f"po{j}")
                        po.append(po_t)
                    for kc in range(KH // KC):
                        wt = wdp.tile([P, KC, ND], bf16, tag="wd")
                        nc.gpsimd.dma_start(
                            wt[:], wdr[:, kc * KC : (kc + 1) * KC, dslice]
                        )
                        for kk in range(KC):
                            k = kc * KC + kk
                            for j in range(8):
                                nc.tensor.matmul(
                                    po[j],
                                    ht[:, k, j * P : (j + 1) * P],
                                    wt[:, kk, :],
                                    start=(k == 0),
                                    stop=(k == KH - 1),
                                )
                    for j in range(8):
                        o = ev.tile([P, ND], f32, tag="ev")
                        nc.vector.tensor_copy(o[:], po[j][:])
                        eng = nc.scalar if j % 2 == 0 else nc.sync
                        eng.dma_start(
                            out[t0 + j * P : t0 + (j + 1) * P, dslice], o[:]
                        )
    nc.compile()
    return nc


def get_nc():
    if "nc" not in _NC_CACHE:
        _NC_CACHE["nc"] = _build_nc()
    return _NC_CACHE["nc"]


def make_in_maps(x, gate_proj, up_proj, down_proj, counts):
    offs = np.concatenate([[0], np.cumsum(counts)])
    in_maps = []
    for e in range(NEXP):
        xe = np.asarray(x[offs[e] : offs[e + 1]], dtype=np.float32)
        if xe.shape[0] < CAP:
            xe = np.concatenate(
                [xe, np.zeros((CAP - xe.shape[0], DIM), np.float32)], axis=0
            )
        in_maps.append(
            {
                "xT": xe.T.astype(BF16, order="C"),
                "wg": np.asarray(gate_proj[e], dtype=np.float32).astype(BF16),
                "wu": np.asarray(up_proj[e], dtype=np.float32).astype(BF16),
                "wd": np.asarray(down_proj[e], dtype=np.float32).astype(BF16),
            }
        )
    return in_maps


def kernel(x, gate_proj, up_proj, down_proj, num_tokens_per_expert, _run_kwargs=None):
    from concourse.bass_utils import run_bass_kernel_spmd

    x = np.asarray(x)
    counts = np.asarray(num_tokens_per_expert).astype(np.int64)
    assert counts.shape[0] == NEXP and counts.max() <= CAP

    in_maps = make_in_maps(x, gate_proj, up_proj, down_proj, counts)
    nc = get_nc()
    res = run_bass_kernel_spmd(
        nc, in_maps, core_ids=list(range(NEXP)), **(_run_kwargs or {})
    )
    outs = [res.results[e]["out"][: counts[e]] for e in range(NEXP)]
    full = np.concatenate(outs, axis=0).astype(np.float32)
    if _run_kwargs is not None:
        _run_kwargs["_results"] = res
    return full


# revision 10
# speedup vs baseline: 1.0815x; 1.0222x over previous
"""Grouped-experts SwiGLU MLP kernel for Trainium2, expert-parallel over 8 cores.

Per core (one expert): out = (silu(x @ Wg) * (x @ Wu)) @ Wd
  x  [2048 tok, 2048 dim]   Wg,Wu [2048 dim, 5632 hid]   Wd [5632 hid, 2048 dim]

Layout trick: host supplies x transposed (xT [dim, tok]).  Phase A computes the
intermediate in transposed layout hT [hid, tok] (stationary = Wg/Wu k-tile,
moving = xT), so phase B can use hT tiles as the stationary operand directly
against Wd (natural layout) and produce out [tok, dim] in natural orientation.
No on-device transposes anywhere.

DMA engine split: gpsimd (SWDGE) carries xT + the Wd stream (prefetchable
during phase A), sync (HWDGE) carries the Wg/Wu stream, scalar (HWDGE) carries
output stores.  One shared 8-bank PSUM pool: phase A keeps 4 gate/up pairs in
flight; phase B accumulates all 8 tok tiles concurrently so Wd streams once
per half.

Numerics: bf16 operands, fp32 PSUM accumulation, fp32 output
(measured end-to-end L2 rel err vs fp32 reference ≈ 4e-3).
"""

import sys

import numpy as np

if "/opt/trn_rl_repo" not in sys.path:
    sys.path.insert(0, "/opt/trn_rl_repo")

import ml_dtypes

P = 128
DIM = 2048
HID = 5632
CAP = 2048  # token capacity per expert
NEXP = 8
KD = DIM // P  # 16 k-tiles over dim
KH = HID // P  # 44 k-tiles over hid
NTOK = 512  # phase A tok group (PSUM bank = 512 fp32)
HALF = 1024  # tok processed per outer iteration
ND = 512  # phase B dim chunk
KC = 11  # phase B k-tiles per Wd DMA chunk (44 = 4*11)

BF16 = ml_dtypes.bfloat16

_NC_CACHE = {}


def _build_nc():
    import concourse.tile as tile
    from concourse import bacc, mybir

    f32 = mybir.dt.float32
    bf16 = mybir.dt.bfloat16
    Silu = mybir.ActivationFunctionType.Silu

    nc = bacc.Bacc("TRN2", target_bir_lowering=False, debug=False)
    xT = nc.dram_tensor("xT", [DIM, CAP], bf16, kind="ExternalInput").ap()
    wg = nc.dram_tensor("wg", [DIM, HID], bf16, kind="ExternalInput").ap()
    wu = nc.dram_tensor("wu", [DIM, HID], bf16, kind="ExternalInput").ap()
    wd = nc.dram_tensor("wd", [HID, DIM], bf16, kind="ExternalInput").ap()
    out = nc.dram_tensor("out", [CAP, DIM], f32, kind="ExternalOutput").ap()

    xTr = xT.rearrange("(ko p) t -> p ko t", p=P)  # [128, 16, 2048]
    wgr = wg.rearrange("(ko p) h -> p ko h", p=P)  # [128, 16, 5632]
    wur = wu.rearrange("(ko p) h -> p ko h", p=P)
    wdr = wd.rearrange("(ko p) d -> p ko d", p=P)  # [128, 44, 2048]

    with tile.TileContext(nc) as tc:
        with (
            tc.tile_pool(name="xp", bufs=2) as xp,
            tc.tile_pool(name="hp", bufs=1) as hp,
            tc.tile_pool(name="wab", bufs=3) as wab,
            tc.tile_pool(name="wdp", bufs=3) as wdp,
            tc.tile_pool(name="tmp", bufs=2) as tmps,
            tc.tile_pool(name="ev", bufs=4) as ev,
            tc.tile_pool(name="ps", bufs=8, space="PSUM") as psp,
        ):
            # HAM warm-up: ~19 us of dependency-free matmuls on zeroed SBUF
            # keep the PE busy (and at full clock) while the first input DMAs
            # land.  Results land in one psum slot and are never read.
            dummy = tmps.tile([P, NTOK], bf16, tag="dummy")
            nc.vector.memset(dummy[:], 0)
            pwarm = psp.tile([P, NTOK], f32, tag="ps", name="pwarm")
            for _ in range(90):
                nc.tensor.matmul(pwarm, dummy[:, :P], dummy[:], start=True, stop=True)

            for half in range(CAP // HALF):
                t0 = half * HALF
                # xT half loaded as two 512-tok tiles so the first matmul
                # group only waits on 2.1 MB (SWDGE queue, ~280 GB/s).
                xts = []
                for tg in range(HALF // NTOK):
                    xt_t = xp.tile([P, KD, NTOK], bf16, tag="xt", name=f"xt{tg}")
                    nc.gpsimd.dma_start(
                        xt_t[:], xTr[:, :, t0 + tg * NTOK : t0 + (tg + 1) * NTOK]
                    )
                    xts.append(xt_t)
                ht = hp.tile([P, KH, HALF], bf16, tag="ht")

                # ---- Phase A: hT[hid, tok_half] = silu(x@Wg)*(x@Wu), transposed
                for m in range(KH):  # 128-wide hid blocks
                    # Wg on the sync HWDGE queue, Wu on the scalar HWDGE queue
                    # (each ~87 GB/s; parallel halves the weight latency chain).
                    g1 = wab.tile([P, KD, P], bf16, tag="wg1")
                    nc.sync.dma_start(g1[:], wgr[:, :, m * P : (m + 1) * P])
                    u1 = wab.tile([P, KD, P], bf16, tag="wu1")
                    nc.scalar.dma_start(u1[:], wur[:, :, m * P : (m + 1) * P])
                    for tg in range(HALF // NTOK):
                        ts_ = slice(tg * NTOK, (tg + 1) * NTOK)
                        pg = psp.tile([P, NTOK], f32, tag="ps", name="pg")
                        pu = psp.tile([P, NTOK], f32, tag="ps", name="pu")
                        for k in range(KD):
                            nc.tensor.matmul(
                                pg,
                                g1[:, k, :],
                                xts[tg][:, k, :],
                                start=(k == 0),
                                stop=(k == KD - 1),
                            )
                        for k in range(KD):
                            nc.tensor.matmul(
                                pu,
                                u1[:, k, :],
                                xts[tg][:, k, :],
                                start=(k == 0),
                                stop=(k == KD - 1),
                            )
                        sl = tmps.tile([P, NTOK], f32, tag="silu")
                        nc.scalar.activation(sl[:], pg[:], Silu)
                        nc.vector.tensor_mul(ht[:, m, ts_], sl[:], pu[:])

                # ---- Phase B: out[tok_half, dim] = hT.T @ Wd
                # 4 tok ptiles accumulate per pass (4 PSUM banks); consecutive
                # passes rotate onto the other 4 banks, so a pass's first
                # matmuls never wait on the previous pass's evacuations.  Wd
                # streams per pass in 1.4 MB chunks on the gpsimd SWDGE queue
                # (~150 GB/s demand vs ~280 available), prefetched via bufs=3.
                for dn in range(DIM // ND):
                    dslice = slice(dn * ND, (dn + 1) * ND)
                    for pset in range(2):
                        po = []
                        for j in range(4):
                            po_t = psp.tile([P, ND], f32, tag="ps", name=f"po{j}")
                            po.append(po_t)
                        for kc in range(KH // KC):
                            wt = wdp.tile([P, KC, ND], bf16, tag="wd")
                            nc.gpsimd.dma_start(
                                wt[:], wdr[:, kc * KC : (kc + 1) * KC, dslice]
                            )
                            for kk in range(KC):
                                k = kc * KC + kk
                                for j in range(4):
                                    tp = pset * 4 + j
                                    nc.tensor.matmul(
                                        po[j],
                                        ht[:, k, tp * P : (tp + 1) * P],
                                        wt[:, kk, :],
                                        start=(k == 0),
                                        stop=(k == KH - 1),
                                    )
                        for j in range(4):
                            tp = pset * 4 + j
                            o = ev.tile([P, ND], f32, tag="ev")
                            nc.vector.tensor_copy(o[:], po[j][:])
                            eng = nc.scalar if j % 2 == 0 else nc.sync
                            eng.dma_start(
                                out[t0 + tp * P : t0 + (tp + 1) * P, dslice], o[:]
                            )
    nc.compile()
    return nc


def get_nc():
    if "nc" not in _NC_CACHE:
        _NC_CACHE["nc"] = _build_nc()
    return _NC_CACHE["nc"]


def make_in_maps(x, gate_proj, up_proj, down_proj, counts):
    offs = np.concatenate([[0], np.cumsum(counts)])
    in_maps = []
    for e in range(NEXP):
        xe = np.asarray(x[offs[e] : offs[e + 1]], dtype=np.float32)
        if xe.shape[0] < CAP:
            xe = np.concatenate(
                [xe, np.zeros((CAP - xe.shape[0], DIM), np.float32)], axis=0
            )
        in_maps.append(
            {
                "xT": xe.T.astype(BF16, order="C"),
                "wg": np.asarray(gate_proj[e], dtype=np.float32).astype(BF16),
                "wu": np.asarray(up_proj[e], dtype=np.float32).astype(BF16),
                "wd": np.asarray(down_proj[e], dtype=np.float32).astype(BF16),
            }
        )
    return in_maps


def kernel(x, gate_proj, up_proj, down_proj, num_tokens_per_expert, _run_kwargs=None):
    from concourse.bass_utils import run_bass_kernel_spmd

    x = np.asarray(x)
    counts = np.asarray(num_tokens_per_expert).astype(np.int64)
    assert counts.shape[0] == NEXP and counts.max() <= CAP

    in_maps = make_in_maps(x, gate_proj, up_proj, down_proj, counts)
    nc = get_nc()
    res = run_bass_kernel_spmd(
        nc, in_maps, core_ids=list(range(NEXP)), **(_run_kwargs or {})
    )
    outs = [res.results[e]["out"][: counts[e]] for e in range(NEXP)]
    full = np.concatenate(outs, axis=0).astype(np.float32)
    if _run_kwargs is not None:
        _run_kwargs["_results"] = res
    return full
